# revision 26
# baseline (speedup 1.0000x reference)
"""Trainium2 Bass kernel for DecouplePreAggGraphConv (GNN message passing).

out[b,j,:] = diag(adj)[j] * (x[b,j] @ W0[j])
           + sum_k offdiag(adj)[j,k] * (x[b,k] @ W1[k])
           + bias

Data-parallel over B across 8 NeuronCores. Per core, per 128-row batch
tile:
  1. one DMA load of x-tile [128, J*128]
  2. PE transposes per joint -> xT_k [n, b] (via identity matmul)
  3. per-joint GEMM  h_k = xT_k.T @ [diag_k*W0_k | W1_k]  -> PSUM [128,256]
  4. drain h to SBUF, then SBUF->SBUF DMA reshuffle into a
     (3-batch-row-group, 35-row) layout: rows = [17 h1 | 17 h0s | bias]
  5. mixing GEMM with a constant block-diagonal [105,51] stationary
     matrix (off.T / I / ones blocks) computes the adjacency mix, the
     self term and the bias add in one pass -> PSUM [51, (g,m)]
  6. drain + one strided store straight into out[b,j,m] layout.
"""

import sys

sys.path.insert(0, "/opt/trn_rl_repo")

import numpy as np

import concourse.bass as bass
import concourse.mybir as mybir
import concourse.tile as tile
from concourse import bacc
from concourse.bass_utils import run_bass_kernel_spmd

B, J, FIN, FOUT = 16384, 17, 128, 128
N_CORES = 8
TB = 128            # batch rows per tile
CJ = J * FOUT       # 2176
G3 = TB // 3        # 42 full groups of 3 rows; rows 126/127 ride as group 42
MAIN = 3 * G3       # 126
NG = G3 + 1         # 43 group slots (last one only has i=0,1 valid)
HPF = NG * FOUT     # 5504 free size of the reshuffled tile
MIXCH = 1024        # mix psum chunk (free elems)
F32 = mybir.dt.float32
BF16 = mybir.dt.bfloat16

_prog_cache: dict[int, object] = {}


def _build_program(bs: int, repeat: int = 1, phases: int = 3):
    """Build the SPMD Bass program for a per-core batch shard of `bs` rows.

    phases (debug/timing only): 1 = stage-1 only, 2 = +bounce, 3 = full.
    """
    nt = bs // TB
    assert bs % TB == 0

    nc = bacc.Bacc("TRN2", target_bir_lowering=False, debug=False,
                   num_devices=N_CORES)

    xs = nc.declare_dram_parameter("xs", [bs, J, FIN], F32, isOutput=False)
    if phases == 0:
        mbig = nc.declare_dram_parameter("mbig", [FIN, J, CJ], F32,
                                         isOutput=False)
        biasv = nc.declare_dram_parameter("biasv", [1, CJ], F32,
                                          isOutput=False)
    else:
        wcat = nc.declare_dram_parameter("wcat", [FIN, J, 2 * FOUT], F32,
                                         isOutput=False)
        mix3 = nc.declare_dram_parameter("mix3", [105, 51], BF16,
                                         isOutput=False)
        bias43 = nc.declare_dram_parameter("bias43", [3, HPF], BF16,
                                           isOutput=False)
    ident = nc.declare_dram_parameter("ident", [128, 128], F32, isOutput=False)
    out = nc.declare_dram_parameter("out", [bs, J, FOUT], F32, isOutput=True)

    if phases == 0:
        return _build_folded(nc, xs, mbig, biasv, ident, out, bs, repeat)

    with tile.TileContext(nc) as tc:
        with (
            tc.tile_pool(name="const", bufs=1) as cpool,
            tc.tile_pool(name="x", bufs=2) as xpool,
            tc.tile_pool(name="xt", bufs=3) as xtpool,
            tc.tile_pool(name="hsb", bufs=2) as hpool,
            tc.tile_pool(name="hp", bufs=2) as hppool,
            tc.tile_pool(name="osb", bufs=2) as opool,
            tc.tile_pool(name="tp", bufs=2, space=bass.MemorySpace.PSUM) as tpp,
            tc.tile_pool(name="hps", bufs=2, space=bass.MemorySpace.PSUM) as hpsp,
            tc.tile_pool(name="mxp", bufs=2, space=bass.MemorySpace.PSUM) as mxpp,
        ):
            # ---- constants, loaded once ----
            wcat_sb = cpool.tile([FIN, J, 2 * FOUT], F32, tag="wcat")
            nc.sync.dma_start(wcat_sb[:], wcat[:])
            mix3_sb = cpool.tile([105, 51], BF16, tag="mix3")
            nc.sync.dma_start(mix3_sb[:], mix3[:])
            id_sb = cpool.tile([128, 128], F32, tag="ident")
            nc.sync.dma_start(id_sb[:], ident[:])

            # ping-pong DRAM scratch for the reshuffle bounce; the
            # (i=2, g=42) rectangle never gets scattered into, so zero it
            # once (PE accumulates 0*garbage = NaN otherwise).
            scrs = [nc.dram_tensor(f"scr{p}", [102, HPF], BF16)
                    for p in range(2)]
            zro = cpool.tile([34, FOUT], BF16, tag="zro")
            nc.gpsimd.memset(zro[:], 0.0)
            for p in range(2):
                nc.sync.dma_start(
                    scrs[p][68:102, G3 * FOUT:], zro[:])

            for t in range(nt * repeat):
                t = t % nt
                b0 = t * TB
                # 1. load x tile
                x_t = xpool.tile([TB, J, FIN], F32, tag="x")
                nc.sync.dma_start(x_t[:], xs[b0:b0 + TB])

                # 2/3/4a. per joint: transpose, GEMM, drain (cast bf16)
                h_sb = hpool.tile([TB, 2, J, FOUT], BF16, tag="h")
                for k in range(J):
                    tp = tpp.tile([128, TB], F32, tag="tp")
                    nc.tensor.transpose(tp[:], x_t[:, k, :], id_sb[:])
                    xt = xtpool.tile([128, TB], F32, tag="xt")
                    if k % 2 == 0:
                        nc.vector.tensor_copy(xt[:], tp[:])
                    else:
                        nc.scalar.copy(xt[:], tp[:])
                    hk = hpsp.tile([TB, 2 * FOUT], F32, tag="hk")
                    nc.tensor.matmul(hk[:], xt[:], wcat_sb[:, k, :])
                    if k % 2 == 0:
                        nc.scalar.copy(h_sb[:, :, k, :], hk[:])
                    else:
                        nc.vector.tensor_copy(h_sb[:, :, k, :], hk[:])

                # 4b. reshuffle via DRAM bounce: scatter h into the group
                # layout in a DRAM scratch (rows r = i*34 + h*17 + k), then
                # read it back contiguously. DRAM APs have no partition-dim
                # restriction, so this is 3 scatter DMAs + 2 readback DMAs.
                if phases == 1:
                    nc.sync.dma_start(
                        out[b0:b0 + TB].rearrange("b j m -> b (j m)")
                        .bitcast(BF16)[:, :J * FOUT],
                        h_sb[:, 0])
                    continue
                scr = scrs[t % 2]
                sv = scr.rearrange("(i h k) (g m) -> i g h k m",
                                   i=3, h=2, k=17, g=NG, m=FOUT)
                for i in range(3):
                    ng = NG if i < 2 else G3
                    nc.sync.dma_start(sv[i, :ng], h_sb[i:TB:3])
                hp_t = hppool.tile([105, HPF], BF16, tag="hp")
                nc.sync.dma_start(hp_t[0:102, :], scr[:])
                nc.sync.dma_start(hp_t[102:105, :], bias43[:])
                if phases == 2:
                    nc.sync.dma_start(
                        out[b0:b0 + 105].rearrange("b j m -> b (j m)")
                        .bitcast(BF16)[:, :43],
                        hp_t[:, 0:43])
                    continue

                # 5/6. mix GEMM chunks, drain, store
                # out[(i,j),(g,m)] = h0s[3g+i,j,m]
                #                  + sum_k off[j,k]*h1[3g+i,k,m] + bias[m]
                # (i=2, g=42) columns are garbage and never stored.
                o_sb = opool.tile([51, HPF], F32, tag="osb")
                nch = (HPF + MIXCH - 1) // MIXCH
                for c in range(nch):
                    f0 = c * MIXCH
                    fw = min(MIXCH, HPF - f0)
                    mp = mxpp.tile([51, MIXCH], F32, tag="mx")
                    for s0 in range(0, fw, 512):
                        sw = min(512, fw - s0)
                        nc.tensor.matmul(mp[:, s0:s0 + sw], mix3_sb[:],
                                         hp_t[:, f0 + s0:f0 + s0 + sw])
                    if c % 2 == 0:
                        nc.vector.tensor_copy(o_sb[:, f0:f0 + fw], mp[:, :fw])
                    else:
                        nc.scalar.copy(o_sb[:, f0:f0 + fw], mp[:, :fw])

                dst = out[b0:b0 + MAIN].rearrange("(g i) j m -> i j g m", i=3)
                nc.sync.dma_start(dst, o_sb[:, :G3 * FOUT])
                nc.sync.dma_start(out[b0 + MAIN:b0 + TB],
                                  o_sb[0:34, G3 * FOUT:])

    nc.compile()
    return nc


def _build_folded(nc, xs, mbig, biasv, ident, out, bs, repeat):
    """Single folded GEMM: out[b,(j,m)] = x[b,(k,n)] @ Mbig + bias.

    Mbig[(k,n),(j,m)] = off[j,k]*W1[k,n,m] + (k==j)*diag[j]*W0[j,n,m].
    2 DMAs per tile; PE streams 17 x 2176 columns per 128-row tile.
    """
    nt = bs // TB
    with tile.TileContext(nc) as tc:
        with (
            tc.tile_pool(name="const", bufs=1) as cpool,
            tc.tile_pool(name="x", bufs=2) as xpool,
            tc.tile_pool(name="xt", bufs=3) as xtpool,
            tc.tile_pool(name="osb", bufs=2) as opool,
            tc.tile_pool(name="tp", bufs=2, space=bass.MemorySpace.PSUM) as tpp,
            tc.tile_pool(name="of", bufs=1, space=bass.MemorySpace.PSUM) as ofp,
        ):
            mb_sb = cpool.tile([FIN, J, CJ], F32, tag="mbig")
            nc.sync.dma_start(mb_sb[:], mbig[:])
            bv_sb = cpool.tile([1, CJ], F32, tag="biasv")
            nc.sync.dma_start(bv_sb[:], biasv[:])
            id_sb = cpool.tile([128, 128], F32, tag="ident")
            nc.sync.dma_start(id_sb[:], ident[:])
            ones = cpool.tile([1, 128], F32, tag="ones")
            nc.gpsimd.memset(ones[:], 1.0)

            chunks = [(c, min(512, CJ - c)) for c in range(0, CJ, 512)]
            for t in range(nt * repeat):
                t = t % nt
                b0 = t * TB
                x_t = xpool.tile([TB, J, FIN], F32, tag="x")
                nc.sync.dma_start(x_t[:], xs[b0:b0 + TB])

                of = ofp.tile([TB, CJ], F32, tag="of")
                for c0, cw in chunks:
                    nc.tensor.matmul(of[:, c0:c0 + cw], ones[:],
                                     bv_sb[:, c0:c0 + cw],
                                     start=True, stop=False)
                for k in range(J):
                    tp = tpp.tile([128, TB], F32, tag="tp")
                    nc.tensor.transpose(tp[:], x_t[:, k, :], id_sb[:])
                    xt = xtpool.tile([128, TB], F32, tag="xt")
                    if k % 2 == 0:
                        nc.vector.tensor_copy(xt[:], tp[:])
                    else:
                        nc.scalar.copy(xt[:], tp[:])
                    for c0, cw in chunks:
                        nc.tensor.matmul(of[:, c0:c0 + cw], xt[:],
                                         mb_sb[:, k, c0:c0 + cw],
                                         start=False, stop=(k == J - 1))

                o_sb = opool.tile([TB, CJ], F32, tag="osb")
                half = CJ // 2
                nc.vector.tensor_copy(o_sb[:, :half], of[:, :half])
                nc.scalar.copy(o_sb[:, half:], of[:, half:])
                nc.sync.dma_start(
                    out[b0:b0 + TB].rearrange("b j m -> b (j m)"), o_sb[:])

    nc.compile()
    return nc


def _host_prep(x, W, bias, adj, bs):
    """Build the per-core input maps."""
    diag = np.diagonal(adj).astype(np.float32)
    off = (adj * (1.0 - np.eye(J, dtype=adj.dtype))).astype(np.float32)

    # stage-1 weights: [FIN, J, 2*FOUT], columns = [diag_k*W0_k | W1_k]
    wcat = np.concatenate([diag[:, None, None] * W[0], W[1]], axis=2)
    wcat = np.ascontiguousarray(wcat.transpose(1, 0, 2)).astype(np.float32)

    # mixing stationary: rows r = i*34 + h*17 + k (h=0: h0s, h=1: h1),
    # rows 102+i: bias; cols (i'*17 + j)
    import ml_dtypes
    mixblock = np.zeros((34, J), dtype=np.float32)
    mixblock[0:J, :] = np.eye(J, dtype=np.float32)  # h0s rows
    mixblock[J:2 * J, :] = off.T      # h1 rows: sum_k off[j,k] h1_k
    mix3 = np.zeros((105, 51), dtype=np.float32)
    for i in range(3):
        mix3[i * 34:(i + 1) * 34, i * J:(i + 1) * J] = mixblock
        mix3[102 + i, i * J:(i + 1) * J] = 1.0      # bias row

    bias43 = np.tile(bias.astype(np.float32), (3, NG))
    ident = np.eye(128, dtype=np.float32)

    # folded weights: Mbig[(k,n),(j,m)], stored n-partition-major
    m4 = off.T[:, :, None, None] * W[1][:, None, :, :]   # [k, j, n, m]
    m4[np.arange(J), np.arange(J)] += diag[:, None, None] * W[0]
    mbig = m4.transpose(0, 2, 1, 3).reshape(J * FIN, CJ)  # rows (k,n)
    mbig = np.ascontiguousarray(
        mbig.reshape(J, FIN, CJ).transpose(1, 0, 2)).astype(np.float32)

    shared = {
        "wcat": wcat,
        "mix3": mix3.astype(ml_dtypes.bfloat16),
        "bias43": np.ascontiguousarray(bias43).astype(ml_dtypes.bfloat16),
        "ident": ident,
        "mbig": mbig,
        "biasv": np.tile(bias.astype(np.float32), 17)[None, :],
    }
    in_maps = []
    for c in range(N_CORES):
        m = dict(shared)
        m["xs"] = np.ascontiguousarray(x[c * bs:(c + 1) * bs])
        in_maps.append(m)
    return in_maps


def _run(x, W, bias, adj, bs, profile=False, tmpdir=None, phases=0):
    key = (bs, phases)
    if key not in _prog_cache:
        _prog_cache[key] = _build_program(bs, phases=phases)
    nc = _prog_cache[key]
    in_maps = _host_prep(x, W, bias, adj, bs)
    res = run_bass_kernel_spmd(nc, in_maps, list(range(N_CORES)),
                               trace=profile, tmpdir=tmpdir)
    out = np.concatenate([res.results[c]["out"] for c in range(N_CORES)],
                         axis=0)
    if profile:
        return out, res
    return out


def kernel(x, W, bias, adj):
    x = np.asarray(x, dtype=np.float32)
    W = np.asarray(W, dtype=np.float32)
    bias = np.asarray(bias, dtype=np.float32)
    adj = np.asarray(adj, dtype=np.float32)
    assert x.shape == (B, J, FIN)
    return _run(x, W, bias, adj, B // N_CORES)


# revision 36
# speedup vs baseline: 3.6094x; 3.6094x over previous
"""Trainium2 Bass kernel for DecouplePreAggGraphConv (GNN message passing).

out[b,j,:] = diag(adj)[j] * (x[b,j] @ W0[j])
           + sum_k offdiag(adj)[j,k] * (x[b,k] @ W1[k])
           + bias

Data-parallel over B across 8 NeuronCores. Per core, per 128-row batch
tile:
  1. one DMA load of x-tile [128, J*128]
  2. PE transposes per joint -> xT_k [n, b] (via identity matmul)
  3. per-joint GEMM  h_k = xT_k.T @ [diag_k*W0_k | W1_k]  -> PSUM [128,256]
  4. drain h to SBUF, then SBUF->SBUF DMA reshuffle into a
     (3-batch-row-group, 35-row) layout: rows = [17 h1 | 17 h0s | bias]
  5. mixing GEMM with a constant block-diagonal [105,51] stationary
     matrix (off.T / I / ones blocks) computes the adjacency mix, the
     self term and the bias add in one pass -> PSUM [51, (g,m)]
  6. drain + one strided store straight into out[b,j,m] layout.
"""

import os
import sys

for _p in ("/opt/trn_rl_repo", "/root/.axon_site/_ro/trn_rl_repo"):
    if os.path.isdir(_p) and _p not in sys.path:
        sys.path.insert(0, _p)

import numpy as np

import concourse.bass as bass
import concourse.mybir as mybir
import concourse.tile as tile
from concourse import bacc
from concourse.bass_utils import run_bass_kernel_spmd

B, J, FIN, FOUT = 16384, 17, 128, 128
N_CORES = 8
TB = 128            # batch rows per tile
CJ = J * FOUT       # 2176
G3 = TB // 3        # 42 full groups of 3 rows; rows 126/127 ride as group 42
MAIN = 3 * G3       # 126
NG = G3 + 1         # 43 group slots (last one only has i=0,1 valid)
HPF = NG * FOUT     # 5504 free size of the reshuffled tile
MIXCH = 1024        # mix psum chunk (free elems)
F32 = mybir.dt.float32
BF16 = mybir.dt.bfloat16

_prog_cache: dict[int, object] = {}


def _build_program(bs: int, repeat: int = 1, phases: int = 3):
    """Build the SPMD Bass program for a per-core batch shard of `bs` rows.

    phases (debug/timing only): 1 = stage-1 only, 2 = +bounce, 3 = full.
    """
    nt = bs // TB
    assert bs % TB == 0

    nc = bacc.Bacc("TRN2", target_bir_lowering=False, debug=False,
                   num_devices=N_CORES)

    xs = nc.declare_dram_parameter("xs", [bs, J, FIN], F32, isOutput=False)
    if phases == 0:
        mbig = nc.declare_dram_parameter("mbig", [FIN, J, CJ],
                                         mybir.dt.float32r, isOutput=False)
        biasv = nc.declare_dram_parameter("biasv", [1, CJ],
                                          mybir.dt.float32r, isOutput=False)
        onesr = nc.declare_dram_parameter("onesr", [1, 128],
                                          mybir.dt.float32r, isOutput=False)
    else:
        wcat = nc.declare_dram_parameter("wcat", [FIN, J, 2 * FOUT], F32,
                                         isOutput=False)
        mix3 = nc.declare_dram_parameter("mix3", [105, 51], BF16,
                                         isOutput=False)
        bias43 = nc.declare_dram_parameter("bias43", [3, HPF], BF16,
                                           isOutput=False)
    ident = nc.declare_dram_parameter("ident", [128, 128], F32, isOutput=False)
    out = nc.declare_dram_parameter("out", [bs, J, FOUT], F32, isOutput=True)

    if phases == 0:
        return _build_folded(nc, xs, mbig, biasv, onesr, ident, out, bs,
                             repeat)

    with tile.TileContext(nc) as tc:
        with (
            tc.tile_pool(name="const", bufs=1) as cpool,
            tc.tile_pool(name="x", bufs=2) as xpool,
            tc.tile_pool(name="xt", bufs=3) as xtpool,
            tc.tile_pool(name="hsb", bufs=2) as hpool,
            tc.tile_pool(name="hp", bufs=2) as hppool,
            tc.tile_pool(name="osb", bufs=2) as opool,
            tc.tile_pool(name="tp", bufs=2, space=bass.MemorySpace.PSUM) as tpp,
            tc.tile_pool(name="hps", bufs=2, space=bass.MemorySpace.PSUM) as hpsp,
            tc.tile_pool(name="mxp", bufs=2, space=bass.MemorySpace.PSUM) as mxpp,
        ):
            # ---- constants, loaded once ----
            wcat_sb = cpool.tile([FIN, J, 2 * FOUT], F32, tag="wcat")
            nc.sync.dma_start(wcat_sb[:], wcat[:])
            mix3_sb = cpool.tile([105, 51], BF16, tag="mix3")
            nc.sync.dma_start(mix3_sb[:], mix3[:])
            id_sb = cpool.tile([128, 128], F32, tag="ident")
            nc.sync.dma_start(id_sb[:], ident[:])

            # ping-pong DRAM scratch for the reshuffle bounce; the
            # (i=2, g=42) rectangle never gets scattered into, so zero it
            # once (PE accumulates 0*garbage = NaN otherwise).
            scrs = [nc.dram_tensor(f"scr{p}", [102, HPF], BF16)
                    for p in range(2)]
            zro = cpool.tile([34, FOUT], BF16, tag="zro")
            nc.gpsimd.memset(zro[:], 0.0)
            for p in range(2):
                nc.sync.dma_start(
                    scrs[p][68:102, G3 * FOUT:], zro[:])

            for t in range(nt * repeat):
                t = t % nt
                b0 = t * TB
                # 1. load x tile
                x_t = xpool.tile([TB, J, FIN], F32, tag="x")
                nc.sync.dma_start(x_t[:], xs[b0:b0 + TB])

                # 2/3/4a. per joint: transpose, GEMM, drain (cast bf16)
                h_sb = hpool.tile([TB, 2, J, FOUT], BF16, tag="h")
                for k in range(J):
                    tp = tpp.tile([128, TB], F32, tag="tp")
                    nc.tensor.transpose(tp[:], x_t[:, k, :], id_sb[:])
                    xt = xtpool.tile([128, TB], F32, tag="xt")
                    if k % 2 == 0:
                        nc.vector.tensor_copy(xt[:], tp[:])
                    else:
                        nc.scalar.copy(xt[:], tp[:])
                    hk = hpsp.tile([TB, 2 * FOUT], F32, tag="hk")
                    nc.tensor.matmul(hk[:], xt[:], wcat_sb[:, k, :])
                    if k % 2 == 0:
                        nc.scalar.copy(h_sb[:, :, k, :], hk[:])
                    else:
                        nc.vector.tensor_copy(h_sb[:, :, k, :], hk[:])

                # 4b. reshuffle via DRAM bounce: scatter h into the group
                # layout in a DRAM scratch (rows r = i*34 + h*17 + k), then
                # read it back contiguously. DRAM APs have no partition-dim
                # restriction, so this is 3 scatter DMAs + 2 readback DMAs.
                if phases == 1:
                    nc.sync.dma_start(
                        out[b0:b0 + TB].rearrange("b j m -> b (j m)")
                        .bitcast(BF16)[:, :J * FOUT],
                        h_sb[:, 0])
                    continue
                scr = scrs[t % 2]
                sv = scr.rearrange("(i h k) (g m) -> i g h k m",
                                   i=3, h=2, k=17, g=NG, m=FOUT)
                for i in range(3):
                    ng = NG if i < 2 else G3
                    nc.sync.dma_start(sv[i, :ng], h_sb[i:TB:3])
                hp_t = hppool.tile([105, HPF], BF16, tag="hp")
                nc.sync.dma_start(hp_t[0:102, :], scr[:])
                nc.sync.dma_start(hp_t[102:105, :], bias43[:])
                if phases == 2:
                    nc.sync.dma_start(
                        out[b0:b0 + 105].rearrange("b j m -> b (j m)")
                        .bitcast(BF16)[:, :43],
                        hp_t[:, 0:43])
                    continue

                # 5/6. mix GEMM chunks, drain, store
                # out[(i,j),(g,m)] = h0s[3g+i,j,m]
                #                  + sum_k off[j,k]*h1[3g+i,k,m] + bias[m]
                # (i=2, g=42) columns are garbage and never stored.
                o_sb = opool.tile([51, HPF], F32, tag="osb")
                nch = (HPF + MIXCH - 1) // MIXCH
                for c in range(nch):
                    f0 = c * MIXCH
                    fw = min(MIXCH, HPF - f0)
                    mp = mxpp.tile([51, MIXCH], F32, tag="mx")
                    for s0 in range(0, fw, 512):
                        sw = min(512, fw - s0)
                        nc.tensor.matmul(mp[:, s0:s0 + sw], mix3_sb[:],
                                         hp_t[:, f0 + s0:f0 + s0 + sw])
                    if c % 2 == 0:
                        nc.vector.tensor_copy(o_sb[:, f0:f0 + fw], mp[:, :fw])
                    else:
                        nc.scalar.copy(o_sb[:, f0:f0 + fw], mp[:, :fw])

                dst = out[b0:b0 + MAIN].rearrange("(g i) j m -> i j g m", i=3)
                nc.sync.dma_start(dst, o_sb[:, :G3 * FOUT])
                nc.sync.dma_start(out[b0 + MAIN:b0 + TB],
                                  o_sb[0:34, G3 * FOUT:])

    nc.compile()
    return nc


def _build_folded(nc, xs, mbig, biasv, onesr, ident, out, bs, repeat):
    """Single folded GEMM: out[b,(j,m)] = x[b,(k,n)] @ Mbig + bias.

    Mbig[(k,n),(j,m)] = off[j,k]*W1[k,n,m] + (k==j)*diag[j]*W0[j,n,m].
    2 DMAs per tile; PE streams 17 x 2176 columns per 128-row tile.
    """
    nt = bs // TB
    with tile.TileContext(nc) as tc:
        with (
            tc.tile_pool(name="const", bufs=1) as cpool,
            tc.tile_pool(name="x", bufs=2) as xpool,
            tc.tile_pool(name="xt", bufs=3) as xtpool,
            tc.tile_pool(name="osb", bufs=2) as opool,
            tc.tile_pool(name="tp", bufs=2, space=bass.MemorySpace.PSUM) as tpp,
            tc.tile_pool(name="of", bufs=1, space=bass.MemorySpace.PSUM) as ofp,
        ):
            F32R = mybir.dt.float32r
            mb_sb = cpool.tile([FIN, J, CJ], F32R, tag="mbig")
            nc.sync.dma_start(mb_sb[:], mbig[:])
            bv_sb = cpool.tile([1, CJ], F32R, tag="biasv")
            nc.sync.dma_start(bv_sb[:], biasv[:])
            id_sb = cpool.tile([128, 128], F32, tag="ident")
            nc.sync.dma_start(id_sb[:], ident[:])
            ones = cpool.tile([1, 128], F32R, tag="ones")
            nc.sync.dma_start(ones[:], onesr[:])

            # fp32 matmul streams at 4 cycles/row on TRN2; float32r (same
            # bits, reduced-precision multiply) streams at 1 cycle/row for
            # N >= 256. Chunks must also stay inside single 2KB PSUM banks
            # (512 f32), so: four aligned 512-wide chunks + a 128 tail.
            chunks = [(0, 512), (512, 512), (1024, 512), (1536, 512),
                      (2048, 128)]
            for t in range(nt * repeat):
                t = t % nt
                b0 = t * TB
                x_t = xpool.tile([TB, J, FIN], F32, tag="x")
                nc.sync.dma_start(x_t[:], xs[b0:b0 + TB])

                of = ofp.tile([TB, CJ], F32, tag="of")
                for c0, cw in chunks:
                    nc.tensor.matmul(of[:, c0:c0 + cw], ones[:],
                                     bv_sb[:, c0:c0 + cw],
                                     start=True, stop=False)
                for k in range(J):
                    tp = tpp.tile([128, TB], F32, tag="tp")
                    nc.tensor.transpose(tp[:], x_t[:, k, :], id_sb[:])
                    xt = xtpool.tile([128, TB], F32R, tag="xt")
                    if k % 2 == 0:
                        nc.vector.tensor_copy(xt[:], tp[:])
                    else:
                        nc.scalar.copy(xt[:], tp[:])
                    for c0, cw in chunks:
                        nc.tensor.matmul(of[:, c0:c0 + cw], xt[:],
                                         mb_sb[:, k, c0:c0 + cw],
                                         start=False, stop=(k == J - 1))

                o_sb = opool.tile([TB, CJ], F32, tag="osb")
                half = CJ // 2
                nc.vector.tensor_copy(o_sb[:, :half], of[:, :half])
                nc.scalar.copy(o_sb[:, half:], of[:, half:])
                nc.sync.dma_start(
                    out[b0:b0 + TB].rearrange("b j m -> b (j m)"), o_sb[:])

    nc.compile()
    return nc


def _host_prep(x, W, bias, adj, bs):
    """Build the per-core input maps."""
    diag = np.diagonal(adj).astype(np.float32)
    off = (adj * (1.0 - np.eye(J, dtype=adj.dtype))).astype(np.float32)

    # stage-1 weights: [FIN, J, 2*FOUT], columns = [diag_k*W0_k | W1_k]
    wcat = np.concatenate([diag[:, None, None] * W[0], W[1]], axis=2)
    wcat = np.ascontiguousarray(wcat.transpose(1, 0, 2)).astype(np.float32)

    # mixing stationary: rows r = i*34 + h*17 + k (h=0: h0s, h=1: h1),
    # rows 102+i: bias; cols (i'*17 + j)
    import ml_dtypes
    mixblock = np.zeros((34, J), dtype=np.float32)
    mixblock[0:J, :] = np.eye(J, dtype=np.float32)  # h0s rows
    mixblock[J:2 * J, :] = off.T      # h1 rows: sum_k off[j,k] h1_k
    mix3 = np.zeros((105, 51), dtype=np.float32)
    for i in range(3):
        mix3[i * 34:(i + 1) * 34, i * J:(i + 1) * J] = mixblock
        mix3[102 + i, i * J:(i + 1) * J] = 1.0      # bias row

    bias43 = np.tile(bias.astype(np.float32), (3, NG))
    ident = np.eye(128, dtype=np.float32)

    # folded weights: Mbig[(k,n),(j,m)], stored n-partition-major
    m4 = off.T[:, :, None, None] * W[1][:, None, :, :]   # [k, j, n, m]
    m4[np.arange(J), np.arange(J)] += diag[:, None, None] * W[0]
    mbig = m4.transpose(0, 2, 1, 3).reshape(J * FIN, CJ)  # rows (k,n)
    mbig = np.ascontiguousarray(
        mbig.reshape(J, FIN, CJ).transpose(1, 0, 2)).astype(np.float32)

    shared = {
        "wcat": wcat,
        "mix3": mix3.astype(ml_dtypes.bfloat16),
        "bias43": np.ascontiguousarray(bias43).astype(ml_dtypes.bfloat16),
        "ident": ident,
        "mbig": mbig,
        "biasv": np.tile(bias.astype(np.float32), 17)[None, :],
        "onesr": np.ones((1, 128), np.float32),
    }
    in_maps = []
    for c in range(N_CORES):
        m = dict(shared)
        m["xs"] = np.ascontiguousarray(x[c * bs:(c + 1) * bs])
        in_maps.append(m)
    return in_maps


def _run(x, W, bias, adj, bs, profile=False, tmpdir=None, phases=0):
    key = (bs, phases)
    if key not in _prog_cache:
        _prog_cache[key] = _build_program(bs, phases=phases)
    nc = _prog_cache[key]
    in_maps = _host_prep(x, W, bias, adj, bs)
    res = run_bass_kernel_spmd(nc, in_maps, list(range(N_CORES)),
                               trace=profile, tmpdir=tmpdir)
    out = np.concatenate([res.results[c]["out"] for c in range(N_CORES)],
                         axis=0)
    if profile:
        return out, res
    return out


def kernel(x, W, bias, adj):
    x = np.asarray(x, dtype=np.float32)
    W = np.asarray(W, dtype=np.float32)
    bias = np.asarray(bias, dtype=np.float32)
    adj = np.asarray(adj, dtype=np.float32)
    assert x.shape == (B, J, FIN)
    return _run(x, W, bias, adj, B // N_CORES)


# revision 39
# speedup vs baseline: 3.9772x; 1.1019x over previous
"""Trainium2 Bass kernel for DecouplePreAggGraphConv (GNN message passing).

out[b,j,:] = diag(adj)[j] * (x[b,j] @ W0[j])
           + sum_k offdiag(adj)[j,k] * (x[b,k] @ W1[k])
           + bias

Data-parallel over B across 8 NeuronCores. Per core, per 128-row batch
tile:
  1. one DMA load of x-tile [128, J*128]
  2. PE transposes per joint -> xT_k [n, b] (via identity matmul)
  3. per-joint GEMM  h_k = xT_k.T @ [diag_k*W0_k | W1_k]  -> PSUM [128,256]
  4. drain h to SBUF, then SBUF->SBUF DMA reshuffle into a
     (3-batch-row-group, 35-row) layout: rows = [17 h1 | 17 h0s | bias]
  5. mixing GEMM with a constant block-diagonal [105,51] stationary
     matrix (off.T / I / ones blocks) computes the adjacency mix, the
     self term and the bias add in one pass -> PSUM [51, (g,m)]
  6. drain + one strided store straight into out[b,j,m] layout.
"""

import os
import sys

for _p in ("/opt/trn_rl_repo", "/root/.axon_site/_ro/trn_rl_repo"):
    if os.path.isdir(_p) and _p not in sys.path:
        sys.path.insert(0, _p)

import numpy as np

import concourse.bass as bass
import concourse.mybir as mybir
import concourse.tile as tile
from concourse import bacc
from concourse.bass_utils import run_bass_kernel_spmd

B, J, FIN, FOUT = 16384, 17, 128, 128
N_CORES = 8
TB = 128            # batch rows per tile
CJ = J * FOUT       # 2176
CJ2 = 2304          # CJ padded to 4.5 PSUM banks so every matmul chunk
                    # is >=256 wide (fp32r full rate) and bank-aligned
G3 = TB // 3        # 42 full groups of 3 rows; rows 126/127 ride as group 42
MAIN = 3 * G3       # 126
NG = G3 + 1         # 43 group slots (last one only has i=0,1 valid)
HPF = NG * FOUT     # 5504 free size of the reshuffled tile
MIXCH = 1024        # mix psum chunk (free elems)
F32 = mybir.dt.float32
BF16 = mybir.dt.bfloat16

_prog_cache: dict[int, object] = {}


def _build_program(bs: int, repeat: int = 1, phases: int = 3):
    """Build the SPMD Bass program for a per-core batch shard of `bs` rows.

    phases (debug/timing only): 1 = stage-1 only, 2 = +bounce, 3 = full.
    """
    nt = bs // TB
    assert bs % TB == 0

    nc = bacc.Bacc("TRN2", target_bir_lowering=False, debug=False,
                   num_devices=N_CORES)

    xs = nc.declare_dram_parameter("xs", [bs, J, FIN], F32, isOutput=False)
    if phases == 0:
        mbig = nc.declare_dram_parameter("mbig", [FIN, J, CJ2],
                                         mybir.dt.float32r, isOutput=False)
        biasv = nc.declare_dram_parameter("biasv", [1, CJ2],
                                          mybir.dt.float32r, isOutput=False)
        onesr = nc.declare_dram_parameter("onesr", [1, 128],
                                          mybir.dt.float32r, isOutput=False)
    else:
        wcat = nc.declare_dram_parameter("wcat", [FIN, J, 2 * FOUT], F32,
                                         isOutput=False)
        mix3 = nc.declare_dram_parameter("mix3", [105, 51], BF16,
                                         isOutput=False)
        bias43 = nc.declare_dram_parameter("bias43", [3, HPF], BF16,
                                           isOutput=False)
    ident = nc.declare_dram_parameter("ident", [128, 128], F32, isOutput=False)
    out = nc.declare_dram_parameter("out", [bs, J, FOUT], F32, isOutput=True)

    if phases == 0:
        return _build_folded(nc, xs, mbig, biasv, onesr, ident, out, bs,
                             repeat)

    with tile.TileContext(nc) as tc:
        with (
            tc.tile_pool(name="const", bufs=1) as cpool,
            tc.tile_pool(name="x", bufs=2) as xpool,
            tc.tile_pool(name="xt", bufs=3) as xtpool,
            tc.tile_pool(name="hsb", bufs=2) as hpool,
            tc.tile_pool(name="hp", bufs=2) as hppool,
            tc.tile_pool(name="osb", bufs=2) as opool,
            tc.tile_pool(name="tp", bufs=2, space=bass.MemorySpace.PSUM) as tpp,
            tc.tile_pool(name="hps", bufs=2, space=bass.MemorySpace.PSUM) as hpsp,
            tc.tile_pool(name="mxp", bufs=2, space=bass.MemorySpace.PSUM) as mxpp,
        ):
            # ---- constants, loaded once ----
            wcat_sb = cpool.tile([FIN, J, 2 * FOUT], F32, tag="wcat")
            nc.sync.dma_start(wcat_sb[:], wcat[:])
            mix3_sb = cpool.tile([105, 51], BF16, tag="mix3")
            nc.sync.dma_start(mix3_sb[:], mix3[:])
            id_sb = cpool.tile([128, 128], F32, tag="ident")
            nc.sync.dma_start(id_sb[:], ident[:])

            # ping-pong DRAM scratch for the reshuffle bounce; the
            # (i=2, g=42) rectangle never gets scattered into, so zero it
            # once (PE accumulates 0*garbage = NaN otherwise).
            scrs = [nc.dram_tensor(f"scr{p}", [102, HPF], BF16)
                    for p in range(2)]
            zro = cpool.tile([34, FOUT], BF16, tag="zro")
            nc.gpsimd.memset(zro[:], 0.0)
            for p in range(2):
                nc.sync.dma_start(
                    scrs[p][68:102, G3 * FOUT:], zro[:])

            for t in range(nt * repeat):
                t = t % nt
                b0 = t * TB
                # 1. load x tile
                x_t = xpool.tile([TB, J, FIN], F32, tag="x")
                nc.sync.dma_start(x_t[:], xs[b0:b0 + TB])

                # 2/3/4a. per joint: transpose, GEMM, drain (cast bf16)
                h_sb = hpool.tile([TB, 2, J, FOUT], BF16, tag="h")
                for k in range(J):
                    tp = tpp.tile([128, TB], F32, tag="tp")
                    nc.tensor.transpose(tp[:], x_t[:, k, :], id_sb[:])
                    xt = xtpool.tile([128, TB], F32, tag="xt")
                    if k % 2 == 0:
                        nc.vector.tensor_copy(xt[:], tp[:])
                    else:
                        nc.scalar.copy(xt[:], tp[:])
                    hk = hpsp.tile([TB, 2 * FOUT], F32, tag="hk")
                    nc.tensor.matmul(hk[:], xt[:], wcat_sb[:, k, :])
                    if k % 2 == 0:
                        nc.scalar.copy(h_sb[:, :, k, :], hk[:])
                    else:
                        nc.vector.tensor_copy(h_sb[:, :, k, :], hk[:])

                # 4b. reshuffle via DRAM bounce: scatter h into the group
                # layout in a DRAM scratch (rows r = i*34 + h*17 + k), then
                # read it back contiguously. DRAM APs have no partition-dim
                # restriction, so this is 3 scatter DMAs + 2 readback DMAs.
                if phases == 1:
                    nc.sync.dma_start(
                        out[b0:b0 + TB].rearrange("b j m -> b (j m)")
                        .bitcast(BF16)[:, :J * FOUT],
                        h_sb[:, 0])
                    continue
                scr = scrs[t % 2]
                sv = scr.rearrange("(i h k) (g m) -> i g h k m",
                                   i=3, h=2, k=17, g=NG, m=FOUT)
                for i in range(3):
                    ng = NG if i < 2 else G3
                    nc.sync.dma_start(sv[i, :ng], h_sb[i:TB:3])
                hp_t = hppool.tile([105, HPF], BF16, tag="hp")
                nc.sync.dma_start(hp_t[0:102, :], scr[:])
                nc.sync.dma_start(hp_t[102:105, :], bias43[:])
                if phases == 2:
                    nc.sync.dma_start(
                        out[b0:b0 + 105].rearrange("b j m -> b (j m)")
                        .bitcast(BF16)[:, :43],
                        hp_t[:, 0:43])
                    continue

                # 5/6. mix GEMM chunks, drain, store
                # out[(i,j),(g,m)] = h0s[3g+i,j,m]
                #                  + sum_k off[j,k]*h1[3g+i,k,m] + bias[m]
                # (i=2, g=42) columns are garbage and never stored.
                o_sb = opool.tile([51, HPF], F32, tag="osb")
                nch = (HPF + MIXCH - 1) // MIXCH
                for c in range(nch):
                    f0 = c * MIXCH
                    fw = min(MIXCH, HPF - f0)
                    mp = mxpp.tile([51, MIXCH], F32, tag="mx")
                    for s0 in range(0, fw, 512):
                        sw = min(512, fw - s0)
                        nc.tensor.matmul(mp[:, s0:s0 + sw], mix3_sb[:],
                                         hp_t[:, f0 + s0:f0 + s0 + sw])
                    if c % 2 == 0:
                        nc.vector.tensor_copy(o_sb[:, f0:f0 + fw], mp[:, :fw])
                    else:
                        nc.scalar.copy(o_sb[:, f0:f0 + fw], mp[:, :fw])

                dst = out[b0:b0 + MAIN].rearrange("(g i) j m -> i j g m", i=3)
                nc.sync.dma_start(dst, o_sb[:, :G3 * FOUT])
                nc.sync.dma_start(out[b0 + MAIN:b0 + TB],
                                  o_sb[0:34, G3 * FOUT:])

    nc.compile()
    return nc


def _build_folded(nc, xs, mbig, biasv, onesr, ident, out, bs, repeat):
    """Single folded GEMM: out[b,(j,m)] = x[b,(k,n)] @ Mbig + bias.

    Mbig[(k,n),(j,m)] = off[j,k]*W1[k,n,m] + (k==j)*diag[j]*W0[j,n,m].
    2 DMAs per tile; PE streams 17 x 2176 columns per 128-row tile.
    """
    nt = bs // TB
    with tile.TileContext(nc) as tc:
        with (
            tc.tile_pool(name="const", bufs=1) as cpool,
            tc.tile_pool(name="x", bufs=2) as xpool,
            tc.tile_pool(name="xt", bufs=3) as xtpool,
            tc.tile_pool(name="osb", bufs=1) as opool,
            tc.tile_pool(name="tp", bufs=2, space=bass.MemorySpace.PSUM) as tpp,
            tc.tile_pool(name="of", bufs=1, space=bass.MemorySpace.PSUM) as ofp,
        ):
            F32R = mybir.dt.float32r
            mb_sb = cpool.tile([FIN, J, CJ2], F32R, tag="mbig")
            nc.sync.dma_start(mb_sb[:], mbig[:])
            bv_sb = cpool.tile([1, CJ2], F32R, tag="biasv")
            nc.sync.dma_start(bv_sb[:], biasv[:])
            id_sb = cpool.tile([128, 128], F32, tag="ident")
            nc.sync.dma_start(id_sb[:], ident[:])
            ones = cpool.tile([1, 128], F32R, tag="ones")
            nc.sync.dma_start(ones[:], onesr[:])

            # fp32 matmul streams at 4 cycles/row on TRN2; float32r (same
            # bits, reduced-precision multiply) streams at 1 cycle/row for
            # N >= 256. Chunks must also stay inside single 2KB PSUM banks
            # (512 f32): four aligned 512-wide chunks + a 256-wide tail
            # into the zero-padded 2176:2304 region.
            chunks = [(0, 512), (512, 512), (1024, 512), (1536, 512),
                      (2048, 256)]
            for t in range(nt * repeat):
                t = t % nt
                b0 = t * TB
                x_t = xpool.tile([TB, J, FIN], F32, tag="x")
                nc.sync.dma_start(x_t[:], xs[b0:b0 + TB])

                of = ofp.tile([TB, CJ2], F32, tag="of")
                for c0, cw in chunks:
                    nc.tensor.matmul(of[:, c0:c0 + cw], ones[:],
                                     bv_sb[:, c0:c0 + cw],
                                     start=True, stop=False)
                for k in range(J):
                    tp = tpp.tile([128, TB], F32, tag="tp")
                    nc.tensor.transpose(tp[:], x_t[:, k, :], id_sb[:])
                    xt = xtpool.tile([128, TB], F32R, tag="xt")
                    if k % 2 == 0:
                        nc.vector.tensor_copy(xt[:], tp[:])
                    else:
                        nc.scalar.copy(xt[:], tp[:])
                    for c0, cw in chunks:
                        nc.tensor.matmul(of[:, c0:c0 + cw], xt[:],
                                         mb_sb[:, k, c0:c0 + cw],
                                         start=False, stop=(k == J - 1))

                o_sb = opool.tile([TB, CJ], F32, tag="osb")
                half = CJ // 2
                nc.vector.tensor_copy(o_sb[:, :half], of[:, :half])
                nc.scalar.copy(o_sb[:, half:], of[:, half:CJ])
                nc.sync.dma_start(
                    out[b0:b0 + TB].rearrange("b j m -> b (j m)"), o_sb[:])

    nc.compile()
    return nc


def _host_prep(x, W, bias, adj, bs):
    """Build the per-core input maps."""
    diag = np.diagonal(adj).astype(np.float32)
    off = (adj * (1.0 - np.eye(J, dtype=adj.dtype))).astype(np.float32)

    # stage-1 weights: [FIN, J, 2*FOUT], columns = [diag_k*W0_k | W1_k]
    wcat = np.concatenate([diag[:, None, None] * W[0], W[1]], axis=2)
    wcat = np.ascontiguousarray(wcat.transpose(1, 0, 2)).astype(np.float32)

    # mixing stationary: rows r = i*34 + h*17 + k (h=0: h0s, h=1: h1),
    # rows 102+i: bias; cols (i'*17 + j)
    import ml_dtypes
    mixblock = np.zeros((34, J), dtype=np.float32)
    mixblock[0:J, :] = np.eye(J, dtype=np.float32)  # h0s rows
    mixblock[J:2 * J, :] = off.T      # h1 rows: sum_k off[j,k] h1_k
    mix3 = np.zeros((105, 51), dtype=np.float32)
    for i in range(3):
        mix3[i * 34:(i + 1) * 34, i * J:(i + 1) * J] = mixblock
        mix3[102 + i, i * J:(i + 1) * J] = 1.0      # bias row

    bias43 = np.tile(bias.astype(np.float32), (3, NG))
    ident = np.eye(128, dtype=np.float32)

    # folded weights: Mbig[(k,n),(j,m)], stored n-partition-major
    m4 = off.T[:, :, None, None] * W[1][:, None, :, :]   # [k, j, n, m]
    m4[np.arange(J), np.arange(J)] += diag[:, None, None] * W[0]
    mbig = m4.transpose(0, 2, 1, 3).reshape(J * FIN, CJ)  # rows (k,n)
    mbig = np.ascontiguousarray(
        mbig.reshape(J, FIN, CJ).transpose(1, 0, 2)).astype(np.float32)
    mbig = np.concatenate(
        [mbig, np.zeros((FIN, J, CJ2 - CJ), np.float32)], axis=2)

    shared = {
        "wcat": wcat,
        "mix3": mix3.astype(ml_dtypes.bfloat16),
        "bias43": np.ascontiguousarray(bias43).astype(ml_dtypes.bfloat16),
        "ident": ident,
        "mbig": mbig,
        "biasv": np.concatenate(
            [np.tile(bias.astype(np.float32), 17),
             np.zeros(CJ2 - CJ, np.float32)])[None, :],
        "onesr": np.ones((1, 128), np.float32),
    }
    in_maps = []
    for c in range(N_CORES):
        m = dict(shared)
        m["xs"] = np.ascontiguousarray(x[c * bs:(c + 1) * bs])
        in_maps.append(m)
    return in_maps


def _run(x, W, bias, adj, bs, profile=False, tmpdir=None, phases=0):
    key = (bs, phases)
    if key not in _prog_cache:
        _prog_cache[key] = _build_program(bs, phases=phases)
    nc = _prog_cache[key]
    in_maps = _host_prep(x, W, bias, adj, bs)
    res = run_bass_kernel_spmd(nc, in_maps, list(range(N_CORES)),
                               trace=profile, tmpdir=tmpdir)
    out = np.concatenate([res.results[c]["out"] for c in range(N_CORES)],
                         axis=0)
    if profile:
        return out, res
    return out


def kernel(x, W, bias, adj):
    x = np.asarray(x, dtype=np.float32)
    W = np.asarray(W, dtype=np.float32)
    bias = np.asarray(bias, dtype=np.float32)
    adj = np.asarray(adj, dtype=np.float32)
    assert x.shape == (B, J, FIN)
    return _run(x, W, bias, adj, B // N_CORES)
